# revision 1
# baseline (speedup 1.0000x reference)
"""Trainium2 Bass kernel for nn_Concentration_61229053772314.

kernel(**inputs) takes the FULL inputs (B=64), shards the batch dim across
8 NeuronCores (pure data parallel, weights replicated), runs a Bass/Tile
kernel via run_bass_kernel_spmd, and reassembles the full outputs.

Self-contained: only imports the concourse runtime that ships with the
environment; does not read any sibling files.
"""
import math
import os
import sys

for _p in ("/opt/trn_rl_repo", "/root/.axon_site/_ro/trn_rl_repo"):
    if os.path.isdir(_p) and _p not in sys.path:
        sys.path.insert(0, _p)

import numpy as np
import concourse.tile as tile
from concourse import bacc, bass_utils, mybir

F32 = mybir.dt.float32
BF16 = mybir.dt.bfloat16
I32 = mybir.dt.int32
U16 = mybir.dt.uint16
AX = mybir.AxisListType
ALU = mybir.AluOpType
ACTF = mybir.ActivationFunctionType

N_CORES = 8
B, A = 64, 32
N = 256    # entries per (b,a)
H = 128    # head dim
K16 = 16   # top-k
GRP = 128  # (b,a) pairs per processing group

NEG_MASK = -1.0e30   # added to masked entries
NEG_REPL = -3.0e38   # match_replace fill (below any real/masked value)

_CACHE = {}


def _build(nc, B_pc):
    NBA = 32 * B_pc
    assert NBA % GRP == 0
    NG = NBA // GRP

    ve_d = nc.dram_tensor("ve", [NBA, N, H], F32, kind="ExternalInput")
    vs_d = nc.dram_tensor("vs", [NBA, H], F32, kind="ExternalInput")
    dead_d = nc.dram_tensor("dead", [NBA, N], I32, kind="ExternalInput")
    wq_d = nc.dram_tensor("wq", [H, H], F32, kind="ExternalInput")
    wk_d = nc.dram_tensor("wk", [H, H], F32, kind="ExternalInput")
    wv_d = nc.dram_tensor("wv", [H, H], F32, kind="ExternalInput")
    wmot_d = nc.dram_tensor("wmot", [H, 2 * H], F32, kind="ExternalInput")
    bmot_d = nc.dram_tensor("bmot", [H, 1], F32, kind="ExternalInput")
    wfwd_d = nc.dram_tensor("wfwd", [H, (K16 + 1) * H], F32, kind="ExternalInput")
    bfwd_d = nc.dram_tensor("bfwd", [H, 1], F32, kind="ExternalInput")
    vc_d = nc.dram_tensor("vc", [NBA, H], F32, kind="ExternalOutput")
    vm_d = nc.dram_tensor("vm", [NBA, H], F32, kind="ExternalOutput")

    with tile.TileContext(nc) as tc:
        _body(nc, tc, NBA, NG, ve_d, vs_d, dead_d, wq_d, wk_d, wv_d,
              wmot_d, bmot_d, wfwd_d, bfwd_d, vc_d, vm_d)


def _body(nc, tc, NBA, NG, ve_d, vs_d, dead_d, wq_d, wk_d, wv_d,
          wmot_d, bmot_d, wfwd_d, bfwd_d, vc_d, vm_d):
    from contextlib import ExitStack
    with ExitStack() as ctx:
        consts = ctx.enter_context(tc.tile_pool(name="consts", bufs=1))
        wpool = ctx.enter_context(tc.tile_pool(name="weights", bufs=1))
        grp_pool = ctx.enter_context(tc.tile_pool(name="grp", bufs=2))
        ve_pool = ctx.enter_context(tc.tile_pool(name="venat", bufs=4))
        vet_pool = ctx.enter_context(tc.tile_pool(name="vet", bufs=4))
        vebf_pool = ctx.enter_context(tc.tile_pool(name="vebf", bufs=160))
        small = ctx.enter_context(tc.tile_pool(name="small", bufs=3))
        ps_vet = ctx.enter_context(tc.tile_pool(name="ps_vet", bufs=2, space="PSUM"))
        ps_cc = ctx.enter_context(tc.tile_pool(name="ps_cc", bufs=2, space="PSUM"))
        ps_xsel = ctx.enter_context(tc.tile_pool(name="ps_xsel", bufs=2, space="PSUM"))
        ps_tr = ctx.enter_context(tc.tile_pool(name="ps_tr", bufs=2, space="PSUM"))

        # constants: iotas -> identities (fp32 + bf16)
        iota_n = consts.tile([128, N], I32)
        nc.gpsimd.iota(iota_n[:], pattern=[[1, N]], base=0, channel_multiplier=0)
        iota_p = consts.tile([128, 1], F32)
        nc.gpsimd.iota(iota_p[:], pattern=[[0, 1]], base=0, channel_multiplier=1,
                       allow_small_or_imprecise_dtypes=True)
        ident_f = consts.tile([128, 128], F32)
        nc.vector.tensor_scalar(ident_f[:], iota_n[:, 0:128], iota_p[:], None,
                                op0=ALU.is_equal)
        ident_b = consts.tile([128, 128], BF16)
        nc.vector.tensor_scalar(ident_b[:], iota_n[:, 0:128], iota_p[:], None,
                                op0=ALU.is_equal)

        # weights
        wq = wpool.tile([H, H], F32)
        nc.sync.dma_start(wq[:], wq_d.ap())
        wk = wpool.tile([H, H], F32)
        nc.sync.dma_start(wk[:], wk_d.ap())
        wv = wpool.tile([H, H], F32)
        nc.sync.dma_start(wv[:], wv_d.ap())
        wmot = wpool.tile([H, 2 * H], F32)
        nc.sync.dma_start(wmot[:], wmot_d.ap())
        wfwd = wpool.tile([H, (K16 + 1) * H], F32)
        nc.sync.dma_start(wfwd[:], wfwd_d.ap())
        bmot = wpool.tile([H, 1], F32)
        nc.sync.dma_start(bmot[:], bmot_d.ap())
        bfwd = wpool.tile([H, 1], F32)
        nc.sync.dma_start(bfwd[:], bfwd_d.ap())

        def pe_transpose_f32(dst_sb, src_sb):
            ps = ps_tr.tile([128, 128], F32, tag="tr")
            nc.tensor.transpose(ps[:], src_sb, ident_f[:])
            nc.scalar.copy(dst_sb, ps[:])

        wkT = wpool.tile([H, H], F32)
        pe_transpose_f32(wkT[:], wk[:])
        wvT = wpool.tile([H, H], F32)
        pe_transpose_f32(wvT[:], wv[:])
        wm0T = wpool.tile([H, H], F32)
        pe_transpose_f32(wm0T[:], wmot[:, 0:H])
        wm1T = wpool.tile([H, H], F32)
        pe_transpose_f32(wm1T[:], wmot[:, H:2 * H])

        # WmvT[iu,o] = sum_i2 WvT[i2,iu] * Wm1T[i2,o]  (= (Wm1 @ Wv^T)^T)
        wmvT_f = wpool.tile([H, H], F32)
        ps = ps_tr.tile([128, 128], F32, tag="tr")
        nc.tensor.matmul(ps[:], wvT[:], wm1T[:])
        nc.scalar.copy(wmvT_f[:], ps[:])

        # W_fwd block transposes -> bf16 [h, ho] blocks packed [128, 17*128]
        wfT_b = wpool.tile([H, (K16 + 1) * H], BF16)
        for j in range(K16 + 1):
            ps = ps_tr.tile([128, 128], F32, tag="tr")
            nc.tensor.transpose(ps[:], wfwd[:, j * H:(j + 1) * H], ident_f[:])
            nc.scalar.copy(wfT_b[:, j * H:(j + 1) * H], ps[:])

        # per-group precompute: VST (vs transposed), T = (Wk @ Wq^T vs)/sqrt(H)
        vst_f, vst_b, t_sb = [], [], []
        for g in range(NG):
            vs_rows = small.tile([GRP, H], F32, tag="vsrows")
            nc.sync.dma_start(vs_rows[:], vs_d.ap()[g * GRP:(g + 1) * GRP, :])
            vstf = grp_pool.tile([H, GRP], F32, tag="vstf")
            pe_transpose_f32(vstf[:], vs_rows[:])
            vstb = grp_pool.tile([H, GRP], BF16, tag="vstb")
            nc.vector.tensor_copy(vstb[:], vstf[:])
            qt = grp_pool.tile([H, GRP], F32, tag="qt")
            ps = ps_tr.tile([128, 128], F32, tag="tr")
            nc.tensor.matmul(ps[:], wq[:], vstf[:])
            nc.scalar.copy(qt[:], ps[:])
            tsb = grp_pool.tile([H, GRP], F32, tag="tsb")
            ps = ps_tr.tile([128, 128], F32, tag="tr")
            nc.tensor.matmul(ps[:], wkT[:], qt[:])
            nc.scalar.mul(tsb[:], ps[:], 1.0 / math.sqrt(H))
            vst_f.append(vstf)
            vst_b.append(vstb)
            t_sb.append(tsb)

        for g in range(NG):
            cc_ps = ps_cc.tile([128, N], F32, tag="cc")
            venat_bf_g = []

            # stage 1: per-ba load, fp32 transpose, bf16 cast, fp32 compat
            for col in range(GRP):
                ib = g * GRP + col
                venat = ve_pool.tile([128, N], F32, tag="venat")
                src = ve_d.ap()[ib].rearrange("(u n) h -> n u h", u=2)
                nc.sync.dma_start(venat[:].rearrange("p (u h) -> p u h", u=2), src)

                veT_ps = ps_vet.tile([128, N], F32, tag="vet")
                nc.tensor.transpose(veT_ps[:, 0:128], venat[:, 0:128], ident_f[:])
                nc.tensor.transpose(veT_ps[:, 128:256], venat[:, 128:256], ident_f[:])
                veT_sb = vet_pool.tile([128, N], F32, tag="vetsb")
                nc.scalar.copy(veT_sb[:], veT_ps[:])

                vebf = vebf_pool.tile([128, N], BF16, tag="vebf")
                nc.any.tensor_copy(vebf[:], venat[:])
                venat_bf_g.append(vebf)

                nc.tensor.matmul(cc_ps[:, col:col + 1], veT_sb[:, 0:128],
                                 t_sb[g][:, col:col + 1])
                nc.tensor.matmul(cc_ps[:, 128 + col:129 + col], veT_sb[:, 128:256],
                                 t_sb[g][:, col:col + 1])

            # stage 2: group softmax + top-16 + one-hot/score transposes
            cc_sb = grp_pool.tile([128, N], F32, tag="ccsb")
            nc.scalar.copy(cc_sb[:], cc_ps[:])
            cmp_ps = ps_tr.tile([128, N], F32, tag="tr")
            nc.tensor.transpose(cmp_ps[:, 0:128], cc_sb[:, 0:128], ident_f[:])
            nc.tensor.transpose(cmp_ps[:, 128:256], cc_sb[:, 128:256], ident_f[:])

            dead_i = grp_pool.tile([128, N], I32, tag="deadi")
            nc.sync.dma_start(dead_i[:], dead_d.ap()[g * GRP:(g + 1) * GRP, :])
            dead_f = grp_pool.tile([128, N], F32, tag="deadf")
            nc.vector.tensor_copy(dead_f[:], dead_i[:])
            cm_sb = grp_pool.tile([128, N], F32, tag="cmsb")
            nc.vector.scalar_tensor_tensor(cm_sb[:], dead_f[:], NEG_MASK,
                                           cmp_ps[:], op0=ALU.mult, op1=ALU.add)

            mx_neg = small.tile([128, 1], F32, tag="mxneg")
            nc.vector.tensor_reduce(mx_neg[:], cm_sb[:], axis=AX.X, op=ALU.max,
                                    negate=True)
            score_un = grp_pool.tile([128, N], F32, tag="scoreun")
            ssum = small.tile([128, 1], F32, tag="ssum")
            nc.scalar.activation(score_un[:], cm_sb[:], ACTF.Exp,
                                 bias=mx_neg[:], scale=1.0, accum_out=ssum[:])
            rs = small.tile([128, 1], F32, tag="rs")
            nc.vector.reciprocal(rs[:], ssum[:])
            score_bf = grp_pool.tile([128, N], BF16, tag="scorebf")
            nc.vector.tensor_scalar_mul(score_bf[:], score_un[:], rs[:])

            mx8a = small.tile([128, 8], F32, tag="mx8a")
            nc.vector.max(mx8a[:], cm_sb[:])
            idx16 = small.tile([128, K16], U16, tag="idx16")
            nc.vector.max_index(idx16[:, 0:8], mx8a[:], cm_sb[:])
            cm2 = grp_pool.tile([128, N], F32, tag="cm2")
            nc.vector.match_replace(cm2[:], mx8a[:], cm_sb[:], NEG_REPL)
            mx8b = small.tile([128, 8], F32, tag="mx8b")
            nc.vector.max(mx8b[:], cm2[:])
            nc.vector.max_index(idx16[:, 8:16], mx8b[:], cm2[:])
            idx_f = small.tile([128, K16], F32, tag="idxbf")
            nc.vector.tensor_copy(idx_f[:], idx16[:])

            s_a = grp_pool.tile([128, GRP * (K16 + 1)], BF16, tag="sa")
            s_b = grp_pool.tile([128, GRP * (K16 + 1)], BF16, tag="sb")
            s_a_v = s_a[:].rearrange("p (b j) -> p b j", j=K16 + 1)
            s_b_v = s_b[:].rearrange("p (b j) -> p b j", j=K16 + 1)
            for j in range(K16):
                sr = grp_pool.tile([128, N], BF16, tag="srj")
                nc.vector.tensor_scalar(sr[:], iota_n[:], idx_f[:, j:j + 1], None,
                                        op0=ALU.is_equal)
                st_ps = ps_tr.tile([128, N], BF16, tag="tr")
                nc.tensor.transpose(st_ps[:, 0:128], sr[:, 0:128], ident_b[:])
                nc.tensor.transpose(st_ps[:, 128:256], sr[:, 128:256], ident_b[:])
                nc.scalar.copy(s_a_v[:, :, j], st_ps[:, 0:128])
                nc.scalar.copy(s_b_v[:, :, j], st_ps[:, 128:256])
            st_ps = ps_tr.tile([128, N], BF16, tag="tr")
            nc.tensor.transpose(st_ps[:, 0:128], score_bf[:, 0:128], ident_b[:])
            nc.tensor.transpose(st_ps[:, 128:256], score_bf[:, 128:256], ident_b[:])
            nc.scalar.copy(s_a_v[:, :, K16], st_ps[:, 0:128])
            nc.scalar.copy(s_b_v[:, :, K16], st_ps[:, 128:256])

            # stage 3: per-ba [gathered rows | u] = ve^T @ [one-hot | score]
            xq_sb = grp_pool.tile([128, GRP * (K16 + 1)], BF16, tag="xq")
            u_f = grp_pool.tile([128, GRP], F32, tag="uf")
            CH = 30
            xsel_ps = None
            for col in range(GRP):
                pos = col % CH
                if pos == 0:
                    xsel_ps = ps_xsel.tile([128, 510], F32, tag="xsel")
                vebf = venat_bf_g[col]
                lo, hi = pos * 17, pos * 17 + 17
                nc.tensor.matmul(xsel_ps[:, lo:hi], vebf[:, 0:128],
                                 s_a[:, col * 17:(col + 1) * 17],
                                 start=True, stop=False)
                nc.tensor.matmul(xsel_ps[:, lo:hi], vebf[:, 128:256],
                                 s_b[:, col * 17:(col + 1) * 17],
                                 start=False, stop=True)
                if pos == CH - 1 or col == GRP - 1:
                    c0 = (col // CH) * CH
                    cnt = col + 1 - c0
                    nc.scalar.copy(xq_sb[:, c0 * 17:(col + 1) * 17],
                                   xsel_ps[:, 0:cnt * 17])
                    xv = xsel_ps[:].rearrange("p (b j) -> p b j", j=K16 + 1)
                    nc.scalar.copy(u_f[:, c0:col + 1], xv[:, 0:cnt, K16])

            xq_v = xq_sb[:].rearrange("p (b j) -> p b j", j=K16 + 1)

            # vC = relu(Wfwd @ [vs | gathered] + bfwd)
            vc_ps = ps_tr.tile([128, GRP], F32, tag="tr")
            nc.tensor.matmul(vc_ps[:], wfT_b[:, 0:H], vst_b[g][:],
                             start=True, stop=False)
            for j in range(1, K16 + 1):
                nc.tensor.matmul(vc_ps[:], wfT_b[:, j * H:(j + 1) * H],
                                 xq_v[:, :, j - 1],
                                 start=False, stop=(j == K16))
            vc_sb = grp_pool.tile([128, GRP], F32, tag="vcsb")
            nc.scalar.activation(vc_sb[:], vc_ps[:], ACTF.Relu,
                                 bias=bfwd[:], scale=1.0)
            vc_tp = ps_tr.tile([128, GRP], F32, tag="tr")
            nc.tensor.transpose(vc_tp[:], vc_sb[:], ident_f[:])
            vc_rows = grp_pool.tile([GRP, H], F32, tag="vcrows")
            nc.scalar.copy(vc_rows[:], vc_tp[:])
            nc.sync.dma_start(vc_d.ap()[g * GRP:(g + 1) * GRP, :], vc_rows[:])

            # vM = relu(Wm0 @ vs + Wmv @ u + bmot)
            vm_ps = ps_tr.tile([128, GRP], F32, tag="tr")
            nc.tensor.matmul(vm_ps[:], wm0T[:], vst_f[g][:],
                             start=True, stop=False)
            nc.tensor.matmul(vm_ps[:], wmvT_f[:], u_f[:],
                             start=False, stop=True)
            vm_sb = grp_pool.tile([128, GRP], F32, tag="vmsb")
            nc.scalar.activation(vm_sb[:], vm_ps[:], ACTF.Relu,
                                 bias=bmot[:], scale=1.0)
            vm_tp = ps_tr.tile([128, GRP], F32, tag="tr")
            nc.tensor.transpose(vm_tp[:], vm_sb[:], ident_f[:])
            vm_rows = grp_pool.tile([GRP, H], F32, tag="vmrows")
            nc.scalar.copy(vm_rows[:], vm_tp[:])
            nc.sync.dma_start(vm_d.ap()[g * GRP:(g + 1) * GRP, :], vm_rows[:])


def _get_compiled(B_pc):
    key = B_pc
    if key not in _CACHE:
        nc = bacc.Bacc("TRN2", target_bir_lowering=False, debug=False,
                       num_devices=N_CORES)
        _build(nc, B_pc)
        nc.compile()
        _CACHE[key] = nc
    return _CACHE[key]


def kernel(vs, ve, ve_dead, Wq, Wk, Wv, W_mot, b_mot, W_fwd, b_fwd,
           trace=False, trace_kwargs=None):
    vs = np.asarray(vs, dtype=np.float32)
    ve = np.asarray(ve, dtype=np.float32)
    ve_dead = np.asarray(ve_dead, dtype=np.int32)
    Bq, Aq = vs.shape[0], vs.shape[1]
    assert (Bq, Aq) == (B, A), (Bq, Aq)
    B_pc = B // N_CORES
    NBA = B_pc * A

    nc = _get_compiled(B_pc)

    shared = {
        "wq": np.ascontiguousarray(Wq, dtype=np.float32),
        "wk": np.ascontiguousarray(Wk, dtype=np.float32),
        "wv": np.ascontiguousarray(Wv, dtype=np.float32),
        "wmot": np.ascontiguousarray(W_mot, dtype=np.float32),
        "bmot": np.ascontiguousarray(b_mot, dtype=np.float32).reshape(H, 1),
        "wfwd": np.ascontiguousarray(W_fwd, dtype=np.float32),
        "bfwd": np.ascontiguousarray(b_fwd, dtype=np.float32).reshape(H, 1),
    }
    in_maps = []
    for c in range(N_CORES):
        sl = slice(c * B_pc, (c + 1) * B_pc)
        in_maps.append({
            "ve": np.ascontiguousarray(ve[sl].reshape(NBA, N, H)),
            "vs": np.ascontiguousarray(vs[sl].reshape(NBA, H)),
            "dead": np.ascontiguousarray(ve_dead[sl].reshape(NBA, N)),
            **shared,
        })

    res = bass_utils.run_bass_kernel_spmd(
        nc, in_maps, core_ids=list(range(N_CORES)),
        trace=trace, **(trace_kwargs or {}))

    vc = np.empty((B, A, H), dtype=np.float32)
    vm = np.empty((B, A, H), dtype=np.float32)
    for c in range(N_CORES):
        sl = slice(c * B_pc, (c + 1) * B_pc)
        vc[sl] = res.results[c]["vc"].reshape(B_pc, A, H)
        vm[sl] = res.results[c]["vm"].reshape(B_pc, A, H)
    kernel.last_results = res
    return (vc, vm)


# revision 16
# speedup vs baseline: 1.4544x; 1.4544x over previous
"""Trainium2 Bass kernel for nn_Concentration_61229053772314.

kernel(**inputs) takes the FULL inputs (B=64), shards the batch dim across
8 NeuronCores (pure data parallel, weights replicated), runs a Bass/Tile
kernel via run_bass_kernel_spmd, and reassembles the full outputs.

Self-contained: only imports the concourse runtime that ships with the
environment; does not read any sibling files.
"""
import math
import os
import sys

for _p in ("/opt/trn_rl_repo", "/root/.axon_site/_ro/trn_rl_repo"):
    if os.path.isdir(_p) and _p not in sys.path:
        sys.path.insert(0, _p)

import numpy as np
import concourse.tile as tile
from concourse import bacc, bass_utils, mybir

F32 = mybir.dt.float32
BF16 = mybir.dt.bfloat16
I32 = mybir.dt.int32
U16 = mybir.dt.uint16
AX = mybir.AxisListType
ALU = mybir.AluOpType
ACTF = mybir.ActivationFunctionType

N_CORES = 8
B, A = 64, 32
N = 256    # entries per (b,a)
H = 128    # head dim
K16 = 16   # top-k
GRP = 128  # (b,a) pairs per processing group

NEG_MASK = -1.0e30   # added to masked entries
NEG_REPL = -3.0e38   # match_replace fill (below any real/masked value)

_CACHE = {}


def _build(nc, B_pc):
    NBA = 32 * B_pc
    assert NBA % GRP == 0
    NG = NBA // GRP

    ve_d = nc.dram_tensor("ve", [NBA, N, H], F32, kind="ExternalInput")
    vs_d = nc.dram_tensor("vs", [NBA, H], F32, kind="ExternalInput")
    dead_d = nc.dram_tensor("dead", [NBA, N], I32, kind="ExternalInput")
    wq_d = nc.dram_tensor("wq", [H, H], F32, kind="ExternalInput")
    wk_d = nc.dram_tensor("wk", [H, H], F32, kind="ExternalInput")
    wv_d = nc.dram_tensor("wv", [H, H], F32, kind="ExternalInput")
    wmot_d = nc.dram_tensor("wmot", [H, 2 * H], F32, kind="ExternalInput")
    bmot_d = nc.dram_tensor("bmot", [H, 1], F32, kind="ExternalInput")
    wfwd_d = nc.dram_tensor("wfwd", [H, (K16 + 1) * H], F32, kind="ExternalInput")
    bfwd_d = nc.dram_tensor("bfwd", [H, 1], F32, kind="ExternalInput")
    vc_d = nc.dram_tensor("vc", [NBA, H], F32, kind="ExternalOutput")
    vm_d = nc.dram_tensor("vm", [NBA, H], F32, kind="ExternalOutput")

    with tile.TileContext(nc) as tc:
        _body(nc, tc, NBA, NG, ve_d, vs_d, dead_d, wq_d, wk_d, wv_d,
              wmot_d, bmot_d, wfwd_d, bfwd_d, vc_d, vm_d)


def _body(nc, tc, NBA, NG, ve_d, vs_d, dead_d, wq_d, wk_d, wv_d,
          wmot_d, bmot_d, wfwd_d, bfwd_d, vc_d, vm_d):
    from contextlib import ExitStack
    with ExitStack() as ctx:
        consts = ctx.enter_context(tc.tile_pool(name="consts", bufs=1))
        wpool = ctx.enter_context(tc.tile_pool(name="weights", bufs=1))
        grp_pool = ctx.enter_context(tc.tile_pool(name="grp", bufs=2))
        ve_pool = ctx.enter_context(tc.tile_pool(name="venat", bufs=4))
        vebf_pool = ctx.enter_context(tc.tile_pool(name="vebf", bufs=160))
        small = ctx.enter_context(tc.tile_pool(name="small", bufs=3))
        ps_tb = ctx.enter_context(tc.tile_pool(name="ps_tb", bufs=3, space="PSUM"))
        dram_pool = ctx.enter_context(tc.tile_pool(name="dram", bufs=2, space="DRAM"))
        ps_xsel = ctx.enter_context(tc.tile_pool(name="ps_xsel", bufs=2, space="PSUM"))
        ps_tr = ctx.enter_context(tc.tile_pool(name="ps_tr", bufs=2, space="PSUM"))

        # constants: iotas -> identities (fp32 + bf16)
        iota_n = consts.tile([128, N], I32)
        nc.gpsimd.iota(iota_n[:], pattern=[[1, N]], base=0, channel_multiplier=0)
        iota_p = consts.tile([128, 1], F32)
        nc.gpsimd.iota(iota_p[:], pattern=[[0, 1]], base=0, channel_multiplier=1,
                       allow_small_or_imprecise_dtypes=True)
        ident_f = consts.tile([128, 128], F32)
        nc.vector.tensor_scalar(ident_f[:], iota_n[:, 0:128], iota_p[:], None,
                                op0=ALU.is_equal)
        ident_b = consts.tile([128, 128], BF16)
        nc.vector.tensor_scalar(ident_b[:], iota_n[:, 0:128], iota_p[:], None,
                                op0=ALU.is_equal)
        ones_all = consts.tile([1, 128], F32)
        nc.gpsimd.memset(ones_all[:], 1.0)

        # weights
        wq = wpool.tile([H, H], F32)
        nc.sync.dma_start(wq[:], wq_d.ap())
        wk = wpool.tile([H, H], F32)
        nc.sync.dma_start(wk[:], wk_d.ap())
        wv = wpool.tile([H, H], F32)
        nc.sync.dma_start(wv[:], wv_d.ap())
        wmot = wpool.tile([H, 2 * H], F32)
        nc.sync.dma_start(wmot[:], wmot_d.ap())
        wfwd = wpool.tile([H, (K16 + 1) * H], F32)
        nc.sync.dma_start(wfwd[:], wfwd_d.ap())
        bmot = wpool.tile([H, 1], F32)
        nc.sync.dma_start(bmot[:], bmot_d.ap())
        bfwd = wpool.tile([H, 1], F32)
        nc.sync.dma_start(bfwd[:], bfwd_d.ap())

        def pe_transpose_f32(dst_sb, src_sb):
            ps = ps_tr.tile([128, 128], F32, tag="tr")
            nc.tensor.transpose(ps[:], src_sb, ident_f[:])
            nc.scalar.copy(dst_sb, ps[:])

        wkT = wpool.tile([H, H], F32)
        pe_transpose_f32(wkT[:], wk[:])
        wvT = wpool.tile([H, H], F32)
        pe_transpose_f32(wvT[:], wv[:])
        wm0T = wpool.tile([H, H], F32)
        pe_transpose_f32(wm0T[:], wmot[:, 0:H])
        wm1T = wpool.tile([H, H], F32)
        pe_transpose_f32(wm1T[:], wmot[:, H:2 * H])

        # WmvT[iu,o] = sum_i2 WvT[i2,iu] * Wm1T[i2,o]  (= (Wm1 @ Wv^T)^T)
        wmvT_f = wpool.tile([H, H], F32)
        ps = ps_tr.tile([128, 128], F32, tag="tr")
        nc.tensor.matmul(ps[:], wvT[:], wm1T[:])
        nc.scalar.copy(wmvT_f[:], ps[:])

        # W_fwd block transposes -> bf16 [h, ho] blocks packed [128, 17*128]
        wfT_b = wpool.tile([H, (K16 + 1) * H], BF16)
        for j in range(K16 + 1):
            ps = ps_tr.tile([128, 128], F32, tag="tr")
            nc.tensor.transpose(ps[:], wfwd[:, j * H:(j + 1) * H], ident_f[:])
            nc.scalar.copy(wfT_b[:, j * H:(j + 1) * H], ps[:])

        # per-group precompute: VST (vs transposed), T = (Wk @ Wq^T vs)/sqrt(H)
        vst_f, vst_b, t_sb = [], [], []
        for g in range(NG):
            vs_rows = small.tile([GRP, H], F32, tag="vsrows")
            nc.sync.dma_start(vs_rows[:], vs_d.ap()[g * GRP:(g + 1) * GRP, :])
            vstf = grp_pool.tile([H, GRP], F32, tag="vstf")
            pe_transpose_f32(vstf[:], vs_rows[:])
            vstb = grp_pool.tile([H, GRP], BF16, tag="vstb")
            nc.vector.tensor_copy(vstb[:], vstf[:])
            qt = grp_pool.tile([H, GRP], F32, tag="qt")
            ps = ps_tr.tile([128, 128], F32, tag="tr")
            nc.tensor.matmul(ps[:], wq[:], vstf[:])
            nc.scalar.copy(qt[:], ps[:])
            tsb = grp_pool.tile([H, GRP], F32, tag="tsb")
            ps = ps_tr.tile([128, 128], F32, tag="tr")
            nc.tensor.matmul(ps[:], wkT[:], qt[:])
            nc.scalar.mul(tsb[:], ps[:], 1.0 / math.sqrt(H))
            # t rows gathered onto partition 0: [1, GRP*H], row ba at [ba*H:(ba+1)*H]
            trows = grp_pool.tile([GRP, H], F32, tag="trows")
            pe_transpose_f32(trows[:], tsb[:])
            t_dram = dram_pool.tile([GRP, H], F32, tag="tdram")
            nc.sync.dma_start(t_dram[:], trows[:])
            vst_f.append(vstf)
            vst_b.append(vstb)
            t_sb.append(t_dram)

        for g in range(NG):
            cc_a = grp_pool.tile([128, GRP], F32, tag="cca")   # [n0-half, ba]
            cc_b = grp_pool.tile([128, GRP], F32, tag="ccb")   # [n1-half, ba]
            venat_bf_g = []
            QB = 4  # ba per DMA batch

            # stage 1: batched load; quad t-broadcast (PE), compat dot (DVE TTR)
            for col in range(GRP):
                ib = g * GRP + col
                if col % 16 == 0:
                    # 16 t-rows onto partition 0 (contiguous in t_dram)
                    t16 = small.tile([1, 16 * H], F32, tag="t16")
                    nc.sync.dma_start(
                        t16[:], t_sb[g][:][col:col + 16, :]
                        .rearrange("b h -> () (b h)"))
                if col % QB == 0:
                    venat4 = ve_pool.tile([128, QB * N], F32, tag="venat")
                    src = ve_d.ap()[ib:ib + QB].rearrange(
                        "b (u n) h -> n b u h", u=2)
                    nc.sync.dma_start(
                        venat4[:].rearrange("p (b u h) -> p b u h", b=QB, u=2), src)
                    # tb4[p, (b,h)] = t_b[h] for the 4 ba of this quad
                    tb4_ps = ps_tb.tile([128, QB * H], F32, tag="tb")
                    qo = (col % 16) * H
                    nc.tensor.matmul(tb4_ps[:], ones_all[:][0:1, :],
                                     t16[:][:, qo:qo + QB * H])
                venat = venat4[:, (col % QB) * N:(col % QB + 1) * N]
                tb = tb4_ps[:, (col % QB) * H:(col % QB + 1) * H]

                scr = small.tile([128, 128], F32, tag="scr")
                nc.vector.scalar_tensor_tensor(
                    scr[:], venat[:, 0:128], 1.0, tb,
                    op0=ALU.mult, op1=ALU.mult, accum_out=cc_a[:, col:col + 1])
                scr2 = small.tile([128, 128], F32, tag="scr2")
                nc.vector.scalar_tensor_tensor(
                    scr2[:], venat[:, 128:256], 1.0, tb,
                    op0=ALU.mult, op1=ALU.mult, accum_out=cc_b[:, col:col + 1])

                vebf = vebf_pool.tile([128, N], BF16, tag="vebf")
                nc.any.tensor_copy(vebf[:], venat[:])
                venat_bf_g.append(vebf)

            # stage 2: group softmax + top-16 + one-hot/score transposes
            cmp_ps = ps_tr.tile([128, N], F32, tag="tr")
            nc.tensor.transpose(cmp_ps[:, 0:128], cc_a[:], ident_f[:])
            nc.tensor.transpose(cmp_ps[:, 128:256], cc_b[:], ident_f[:])

            dead_i = grp_pool.tile([128, N], I32, tag="deadi")
            nc.sync.dma_start(dead_i[:], dead_d.ap()[g * GRP:(g + 1) * GRP, :])
            dead_f = grp_pool.tile([128, N], F32, tag="deadf")
            nc.vector.tensor_copy(dead_f[:], dead_i[:])
            cm_sb = grp_pool.tile([128, N], F32, tag="cmsb")
            nc.vector.scalar_tensor_tensor(cm_sb[:], dead_f[:], NEG_MASK,
                                           cmp_ps[:], op0=ALU.mult, op1=ALU.add)

            mx_neg = small.tile([128, 1], F32, tag="mxneg")
            nc.vector.tensor_reduce(mx_neg[:], cm_sb[:], axis=AX.X, op=ALU.max,
                                    negate=True)
            score_un = grp_pool.tile([128, N], F32, tag="scoreun")
            ssum = small.tile([128, 1], F32, tag="ssum")
            nc.scalar.activation(score_un[:], cm_sb[:], ACTF.Exp,
                                 bias=mx_neg[:], scale=1.0, accum_out=ssum[:])
            rs = small.tile([128, 1], F32, tag="rs")
            nc.vector.reciprocal(rs[:], ssum[:])
            score_bf = grp_pool.tile([128, N], BF16, tag="scorebf")
            nc.vector.tensor_scalar_mul(score_bf[:], score_un[:], rs[:])

            mx8a = small.tile([128, 8], F32, tag="mx8a")
            nc.vector.max(mx8a[:], cm_sb[:])
            idx16 = small.tile([128, K16], U16, tag="idx16")
            nc.vector.max_index(idx16[:, 0:8], mx8a[:], cm_sb[:])
            cm2 = grp_pool.tile([128, N], F32, tag="cm2")
            nc.vector.match_replace(cm2[:], mx8a[:], cm_sb[:], NEG_REPL)
            mx8b = small.tile([128, 8], F32, tag="mx8b")
            nc.vector.max(mx8b[:], cm2[:])
            nc.vector.max_index(idx16[:, 8:16], mx8b[:], cm2[:])
            idx_f = small.tile([128, K16], F32, tag="idxbf")
            nc.vector.tensor_copy(idx_f[:], idx16[:])

            s_a = grp_pool.tile([128, GRP * (K16 + 1)], BF16, tag="sa")
            s_b = grp_pool.tile([128, GRP * (K16 + 1)], BF16, tag="sb")
            s_a_v = s_a[:].rearrange("p (b j) -> p b j", j=K16 + 1)
            s_b_v = s_b[:].rearrange("p (b j) -> p b j", j=K16 + 1)
            for j in range(K16):
                sr = grp_pool.tile([128, N], BF16, tag="srj")
                nc.vector.tensor_scalar(sr[:], iota_n[:], idx_f[:, j:j + 1], None,
                                        op0=ALU.is_equal)
                st_ps = ps_tr.tile([128, N], BF16, tag="tr")
                nc.tensor.transpose(st_ps[:, 0:128], sr[:, 0:128], ident_b[:])
                nc.tensor.transpose(st_ps[:, 128:256], sr[:, 128:256], ident_b[:])
                nc.scalar.copy(s_a_v[:, :, j], st_ps[:, 0:128])
                nc.scalar.copy(s_b_v[:, :, j], st_ps[:, 128:256])
            st_ps = ps_tr.tile([128, N], BF16, tag="tr")
            nc.tensor.transpose(st_ps[:, 0:128], score_bf[:, 0:128], ident_b[:])
            nc.tensor.transpose(st_ps[:, 128:256], score_bf[:, 128:256], ident_b[:])
            nc.scalar.copy(s_a_v[:, :, K16], st_ps[:, 0:128])
            nc.scalar.copy(s_b_v[:, :, K16], st_ps[:, 128:256])

            # stage 3: per-ba [gathered rows | u] = ve^T @ [one-hot | score]
            xq_sb = grp_pool.tile([128, GRP * (K16 + 1)], BF16, tag="xq")
            u_f = grp_pool.tile([128, GRP], F32, tag="uf")
            CH = 30
            xsel_ps = None
            for col in range(GRP):
                pos = col % CH
                if pos == 0:
                    xsel_ps = ps_xsel.tile([128, 510], F32, tag="xsel")
                vebf = venat_bf_g[col]
                lo, hi = pos * 17, pos * 17 + 17
                nc.tensor.matmul(xsel_ps[:, lo:hi], vebf[:, 0:128],
                                 s_a[:, col * 17:(col + 1) * 17],
                                 start=True, stop=False)
                nc.tensor.matmul(xsel_ps[:, lo:hi], vebf[:, 128:256],
                                 s_b[:, col * 17:(col + 1) * 17],
                                 start=False, stop=True)
                if pos == CH - 1 or col == GRP - 1:
                    c0 = (col // CH) * CH
                    cnt = col + 1 - c0
                    nc.scalar.copy(xq_sb[:, c0 * 17:(col + 1) * 17],
                                   xsel_ps[:, 0:cnt * 17])
                    xv = xsel_ps[:].rearrange("p (b j) -> p b j", j=K16 + 1)
                    nc.scalar.copy(u_f[:, c0:col + 1], xv[:, 0:cnt, K16])

            xq_v = xq_sb[:].rearrange("p (b j) -> p b j", j=K16 + 1)

            # vC = relu(Wfwd @ [vs | gathered] + bfwd)
            vc_ps = ps_tr.tile([128, GRP], F32, tag="tr")
            nc.tensor.matmul(vc_ps[:], wfT_b[:, 0:H], vst_b[g][:],
                             start=True, stop=False)
            for j in range(1, K16 + 1):
                nc.tensor.matmul(vc_ps[:], wfT_b[:, j * H:(j + 1) * H],
                                 xq_v[:, :, j - 1],
                                 start=False, stop=(j == K16))
            vc_sb = grp_pool.tile([128, GRP], F32, tag="vcsb")
            nc.scalar.activation(vc_sb[:], vc_ps[:], ACTF.Relu,
                                 bias=bfwd[:], scale=1.0)
            vc_tp = ps_tr.tile([128, GRP], F32, tag="tr")
            nc.tensor.transpose(vc_tp[:], vc_sb[:], ident_f[:])
            vc_rows = grp_pool.tile([GRP, H], F32, tag="vcrows")
            nc.scalar.copy(vc_rows[:], vc_tp[:])
            nc.sync.dma_start(vc_d.ap()[g * GRP:(g + 1) * GRP, :], vc_rows[:])

            # vM = relu(Wm0 @ vs + Wmv @ u + bmot)
            vm_ps = ps_tr.tile([128, GRP], F32, tag="tr")
            nc.tensor.matmul(vm_ps[:], wm0T[:], vst_f[g][:],
                             start=True, stop=False)
            nc.tensor.matmul(vm_ps[:], wmvT_f[:], u_f[:],
                             start=False, stop=True)
            vm_sb = grp_pool.tile([128, GRP], F32, tag="vmsb")
            nc.scalar.activation(vm_sb[:], vm_ps[:], ACTF.Relu,
                                 bias=bmot[:], scale=1.0)
            vm_tp = ps_tr.tile([128, GRP], F32, tag="tr")
            nc.tensor.transpose(vm_tp[:], vm_sb[:], ident_f[:])
            vm_rows = grp_pool.tile([GRP, H], F32, tag="vmrows")
            nc.scalar.copy(vm_rows[:], vm_tp[:])
            nc.sync.dma_start(vm_d.ap()[g * GRP:(g + 1) * GRP, :], vm_rows[:])


def _get_compiled(B_pc):
    key = B_pc
    if key not in _CACHE:
        nc = bacc.Bacc("TRN2", target_bir_lowering=False, debug=False,
                       num_devices=N_CORES)
        _build(nc, B_pc)
        nc.compile()
        _CACHE[key] = nc
    return _CACHE[key]


def kernel(vs, ve, ve_dead, Wq, Wk, Wv, W_mot, b_mot, W_fwd, b_fwd,
           trace=False, trace_kwargs=None):
    vs = np.asarray(vs, dtype=np.float32)
    ve = np.asarray(ve, dtype=np.float32)
    ve_dead = np.asarray(ve_dead, dtype=np.int32)
    Bq, Aq = vs.shape[0], vs.shape[1]
    assert (Bq, Aq) == (B, A), (Bq, Aq)
    B_pc = B // N_CORES
    NBA = B_pc * A

    nc = _get_compiled(B_pc)

    shared = {
        "wq": np.ascontiguousarray(Wq, dtype=np.float32),
        "wk": np.ascontiguousarray(Wk, dtype=np.float32),
        "wv": np.ascontiguousarray(Wv, dtype=np.float32),
        "wmot": np.ascontiguousarray(W_mot, dtype=np.float32),
        "bmot": np.ascontiguousarray(b_mot, dtype=np.float32).reshape(H, 1),
        "wfwd": np.ascontiguousarray(W_fwd, dtype=np.float32),
        "bfwd": np.ascontiguousarray(b_fwd, dtype=np.float32).reshape(H, 1),
    }
    in_maps = []
    for c in range(N_CORES):
        sl = slice(c * B_pc, (c + 1) * B_pc)
        in_maps.append({
            "ve": np.ascontiguousarray(ve[sl].reshape(NBA, N, H)),
            "vs": np.ascontiguousarray(vs[sl].reshape(NBA, H)),
            "dead": np.ascontiguousarray(ve_dead[sl].reshape(NBA, N)),
            **shared,
        })

    res = bass_utils.run_bass_kernel_spmd(
        nc, in_maps, core_ids=list(range(N_CORES)),
        trace=trace, **(trace_kwargs or {}))

    vc = np.empty((B, A, H), dtype=np.float32)
    vm = np.empty((B, A, H), dtype=np.float32)
    for c in range(N_CORES):
        sl = slice(c * B_pc, (c + 1) * B_pc)
        vc[sl] = res.results[c]["vc"].reshape(B_pc, A, H)
        vm[sl] = res.results[c]["vm"].reshape(B_pc, A, H)
    kernel.last_results = res
    return (vc, vm)


# revision 19
# speedup vs baseline: 1.7939x; 1.2335x over previous
"""Trainium2 Bass kernel for nn_Concentration_61229053772314.

kernel(**inputs) takes the FULL inputs (B=64), shards the batch dim across
8 NeuronCores (pure data parallel, weights replicated), runs a Bass/Tile
kernel via run_bass_kernel_spmd, and reassembles the full outputs.

Self-contained: only imports the concourse runtime that ships with the
environment; does not read any sibling files.
"""
import math
import os
import sys

for _p in ("/opt/trn_rl_repo", "/root/.axon_site/_ro/trn_rl_repo"):
    if os.path.isdir(_p) and _p not in sys.path:
        sys.path.insert(0, _p)

import numpy as np
import concourse.tile as tile
from concourse import bacc, bass_utils, mybir

F32 = mybir.dt.float32
BF16 = mybir.dt.bfloat16
I32 = mybir.dt.int32
U16 = mybir.dt.uint16
AX = mybir.AxisListType
ALU = mybir.AluOpType
ACTF = mybir.ActivationFunctionType

N_CORES = 8
B, A = 64, 32
N = 256    # entries per (b,a)
H = 128    # head dim
K16 = 16   # top-k
GRP = 128  # (b,a) pairs per processing group

NEG_MASK = -1.0e30   # added to masked entries
NEG_REPL = -3.0e38   # match_replace fill (below any real/masked value)

_CACHE = {}


def _build(nc, B_pc):
    NBA = 32 * B_pc
    assert NBA % GRP == 0
    NG = NBA // GRP

    ve_d = nc.dram_tensor("ve", [NBA, N, H], F32, kind="ExternalInput")
    vs_d = nc.dram_tensor("vs", [NBA, H], F32, kind="ExternalInput")
    dead_d = nc.dram_tensor("dead", [NBA, N], I32, kind="ExternalInput")
    wq_d = nc.dram_tensor("wq", [H, H], F32, kind="ExternalInput")
    wk_d = nc.dram_tensor("wk", [H, H], F32, kind="ExternalInput")
    wv_d = nc.dram_tensor("wv", [H, H], F32, kind="ExternalInput")
    wmot_d = nc.dram_tensor("wmot", [H, 2 * H], F32, kind="ExternalInput")
    bmot_d = nc.dram_tensor("bmot", [H, 1], F32, kind="ExternalInput")
    wfwd_d = nc.dram_tensor("wfwd", [H, (K16 + 1) * H], F32, kind="ExternalInput")
    bfwd_d = nc.dram_tensor("bfwd", [H, 1], F32, kind="ExternalInput")
    vc_d = nc.dram_tensor("vc", [NBA, H], F32, kind="ExternalOutput")
    vm_d = nc.dram_tensor("vm", [NBA, H], F32, kind="ExternalOutput")

    with tile.TileContext(nc) as tc:
        _body(nc, tc, NBA, NG, ve_d, vs_d, dead_d, wq_d, wk_d, wv_d,
              wmot_d, bmot_d, wfwd_d, bfwd_d, vc_d, vm_d)


def _body(nc, tc, NBA, NG, ve_d, vs_d, dead_d, wq_d, wk_d, wv_d,
          wmot_d, bmot_d, wfwd_d, bfwd_d, vc_d, vm_d):
    from contextlib import ExitStack
    with ExitStack() as ctx:
        consts = ctx.enter_context(tc.tile_pool(name="consts", bufs=1))
        wpool = ctx.enter_context(tc.tile_pool(name="weights", bufs=1))
        grp_pool = ctx.enter_context(tc.tile_pool(name="grp", bufs=2))
        ve_pool = ctx.enter_context(tc.tile_pool(name="venat", bufs=4))
        vebf_pool = ctx.enter_context(tc.tile_pool(name="vebf", bufs=160))
        small = ctx.enter_context(tc.tile_pool(name="small", bufs=3))
        dram_pool = ctx.enter_context(tc.tile_pool(name="dram", bufs=2, space="DRAM"))
        ps_xsel = ctx.enter_context(tc.tile_pool(name="ps_xsel", bufs=2, space="PSUM"))
        ps_tr = ctx.enter_context(tc.tile_pool(name="ps_tr", bufs=3, space="PSUM"))

        # constants: iotas -> identities (fp32 + bf16)
        iota_n = consts.tile([128, N], I32)
        nc.gpsimd.iota(iota_n[:], pattern=[[1, N]], base=0, channel_multiplier=0)
        iota_p = consts.tile([128, 1], F32)
        nc.gpsimd.iota(iota_p[:], pattern=[[0, 1]], base=0, channel_multiplier=1,
                       allow_small_or_imprecise_dtypes=True)
        ident_f = consts.tile([128, 128], F32)
        nc.vector.tensor_scalar(ident_f[:], iota_n[:, 0:128], iota_p[:], None,
                                op0=ALU.is_equal)
        ident_b = consts.tile([128, 128], BF16)
        nc.vector.tensor_scalar(ident_b[:], iota_n[:, 0:128], iota_p[:], None,
                                op0=ALU.is_equal)
        ones_all = consts.tile([1, 128], F32)
        nc.gpsimd.memset(ones_all[:], 1.0)

        # weights
        wq = wpool.tile([H, H], F32)
        nc.sync.dma_start(wq[:], wq_d.ap())
        wk = wpool.tile([H, H], F32)
        nc.sync.dma_start(wk[:], wk_d.ap())
        wv = wpool.tile([H, H], F32)
        nc.sync.dma_start(wv[:], wv_d.ap())
        wmot = wpool.tile([H, 2 * H], F32)
        nc.sync.dma_start(wmot[:], wmot_d.ap())
        wfwd = wpool.tile([H, (K16 + 1) * H], F32)
        nc.sync.dma_start(wfwd[:], wfwd_d.ap())
        bmot = wpool.tile([H, 1], F32)
        nc.sync.dma_start(bmot[:], bmot_d.ap())
        bfwd = wpool.tile([H, 1], F32)
        nc.sync.dma_start(bfwd[:], bfwd_d.ap())

        def pe_transpose_f32(dst_sb, src_sb):
            ps = ps_tr.tile([128, 128], F32, tag="tr")
            nc.tensor.transpose(ps[:], src_sb, ident_f[:])
            nc.scalar.copy(dst_sb, ps[:])

        wkT = wpool.tile([H, H], F32)
        pe_transpose_f32(wkT[:], wk[:])
        wvT = wpool.tile([H, H], F32)
        pe_transpose_f32(wvT[:], wv[:])
        wm0T = wpool.tile([H, H], F32)
        pe_transpose_f32(wm0T[:], wmot[:, 0:H])
        wm1T = wpool.tile([H, H], F32)
        pe_transpose_f32(wm1T[:], wmot[:, H:2 * H])

        # WmvT[iu,o] = sum_i2 WvT[i2,iu] * Wm1T[i2,o]  (= (Wm1 @ Wv^T)^T)
        wmvT_f = wpool.tile([H, H], F32)
        ps = ps_tr.tile([128, 128], F32, tag="tr")
        nc.tensor.matmul(ps[:], wvT[:], wm1T[:])
        nc.scalar.copy(wmvT_f[:], ps[:])

        # W_fwd block transposes -> bf16 [h, ho] blocks packed [128, 17*128]
        wfT_b = wpool.tile([H, (K16 + 1) * H], BF16)
        for j in range(K16 + 1):
            ps = ps_tr.tile([128, 128], F32, tag="tr")
            nc.tensor.transpose(ps[:], wfwd[:, j * H:(j + 1) * H], ident_f[:])
            nc.scalar.copy(wfT_b[:, j * H:(j + 1) * H], ps[:])

        # per-group precompute: VST (vs transposed), T = (Wk @ Wq^T vs)/sqrt(H)
        vst_f, vst_b, t_sb = [], [], []
        for g in range(NG):
            vs_rows = small.tile([GRP, H], F32, tag="vsrows")
            nc.sync.dma_start(vs_rows[:], vs_d.ap()[g * GRP:(g + 1) * GRP, :])
            vstf = grp_pool.tile([H, GRP], F32, tag="vstf")
            pe_transpose_f32(vstf[:], vs_rows[:])
            vstb = grp_pool.tile([H, GRP], BF16, tag="vstb")
            nc.vector.tensor_copy(vstb[:], vstf[:])
            qt = grp_pool.tile([H, GRP], F32, tag="qt")
            ps = ps_tr.tile([128, 128], F32, tag="tr")
            nc.tensor.matmul(ps[:], wq[:], vstf[:])
            nc.scalar.copy(qt[:], ps[:])
            tsb = grp_pool.tile([H, GRP], F32, tag="tsb")
            ps = ps_tr.tile([128, 128], F32, tag="tr")
            nc.tensor.matmul(ps[:], wkT[:], qt[:])
            nc.scalar.mul(tsb[:], ps[:], 1.0 / math.sqrt(H))
            # t rows gathered onto partition 0: [1, GRP*H], row ba at [ba*H:(ba+1)*H]
            trows = grp_pool.tile([GRP, H], F32, tag="trows")
            pe_transpose_f32(trows[:], tsb[:])
            t_dram = dram_pool.tile([GRP, H], F32, tag="tdram")
            nc.sync.dma_start(t_dram[:], trows[:])
            vst_f.append(vstf)
            vst_b.append(vstb)
            t_sb.append(t_dram)

        for g in range(NG):
            cc_a = grp_pool.tile([128, GRP], F32, tag="cca")   # [n0-half, ba]
            cc_b = grp_pool.tile([128, GRP], F32, tag="ccb")   # [n1-half, ba]
            venat_bf_g = []
            QB = 8  # ba per DMA batch

            # stage 1: batched load; t DMA-broadcast to all partitions; DVE dot
            for col in range(GRP):
                ib = g * GRP + col
                if col % 16 == 0:
                    # t rows for 16 ba, replicated across all 128 partitions
                    tbs16 = small.tile([128, 16 * H], F32, tag="t16")
                    nc.sync.dma_start(
                        tbs16[:], t_sb[g][:][col:col + 16, :]
                        .rearrange("b h -> (b h)").partition_broadcast(128))
                if col % QB == 0:
                    venat4 = ve_pool.tile([128, QB * N], F32, tag="venat")
                    src = ve_d.ap()[ib:ib + QB].rearrange(
                        "b (u n) h -> n b u h", u=2)
                    nc.sync.dma_start(
                        venat4[:].rearrange("p (b u h) -> p b u h", b=QB, u=2), src)
                venat = venat4[:, (col % QB) * N:(col % QB + 1) * N]
                tb = tbs16[:, (col % 16) * H:(col % 16 + 1) * H]

                scr = small.tile([128, 128], F32, tag="scr")
                nc.vector.scalar_tensor_tensor(
                    scr[:], venat[:, 0:128], 1.0, tb,
                    op0=ALU.mult, op1=ALU.mult, accum_out=cc_a[:, col:col + 1])
                scr2 = small.tile([128, 128], F32, tag="scr2")
                nc.vector.scalar_tensor_tensor(
                    scr2[:], venat[:, 128:256], 1.0, tb,
                    op0=ALU.mult, op1=ALU.mult, accum_out=cc_b[:, col:col + 1])

                vebf = vebf_pool.tile([128, N], BF16, tag="vebf")
                nc.any.tensor_copy(vebf[:], venat[:])
                venat_bf_g.append(vebf)

            # stage 2: group softmax + top-16 + one-hot/score transposes
            cmp_ps = ps_tr.tile([128, N], F32, tag="tr")
            nc.tensor.transpose(cmp_ps[:, 0:128], cc_a[:], ident_f[:])
            nc.tensor.transpose(cmp_ps[:, 128:256], cc_b[:], ident_f[:])

            dead_i = grp_pool.tile([128, N], I32, tag="deadi")
            nc.sync.dma_start(dead_i[:], dead_d.ap()[g * GRP:(g + 1) * GRP, :])
            dead_f = grp_pool.tile([128, N], F32, tag="deadf")
            nc.vector.tensor_copy(dead_f[:], dead_i[:])
            cm_sb = grp_pool.tile([128, N], F32, tag="cmsb")
            nc.vector.scalar_tensor_tensor(cm_sb[:], dead_f[:], NEG_MASK,
                                           cmp_ps[:], op0=ALU.mult, op1=ALU.add)

            mx_neg = small.tile([128, 1], F32, tag="mxneg")
            nc.vector.tensor_reduce(mx_neg[:], cm_sb[:], axis=AX.X, op=ALU.max,
                                    negate=True)
            score_un = grp_pool.tile([128, N], F32, tag="scoreun")
            ssum = small.tile([128, 1], F32, tag="ssum")
            nc.scalar.activation(score_un[:], cm_sb[:], ACTF.Exp,
                                 bias=mx_neg[:], scale=1.0, accum_out=ssum[:])
            rs = small.tile([128, 1], F32, tag="rs")
            nc.vector.reciprocal(rs[:], ssum[:])
            score_bf = grp_pool.tile([128, N], BF16, tag="scorebf")
            nc.vector.tensor_scalar_mul(score_bf[:], score_un[:], rs[:])

            mx8a = small.tile([128, 8], F32, tag="mx8a")
            nc.vector.max(mx8a[:], cm_sb[:])
            idx16 = small.tile([128, K16], U16, tag="idx16")
            nc.vector.max_index(idx16[:, 0:8], mx8a[:], cm_sb[:])
            cm2 = grp_pool.tile([128, N], F32, tag="cm2")
            nc.vector.match_replace(cm2[:], mx8a[:], cm_sb[:], NEG_REPL)
            mx8b = small.tile([128, 8], F32, tag="mx8b")
            nc.vector.max(mx8b[:], cm2[:])
            nc.vector.max_index(idx16[:, 8:16], mx8b[:], cm2[:])
            idx_f = small.tile([128, K16], F32, tag="idxbf")
            nc.vector.tensor_copy(idx_f[:], idx16[:])

            s_a = grp_pool.tile([128, GRP * (K16 + 1)], BF16, tag="sa")
            s_b = grp_pool.tile([128, GRP * (K16 + 1)], BF16, tag="sb")
            s_a_v = s_a[:].rearrange("p (b j) -> p b j", j=K16 + 1)
            s_b_v = s_b[:].rearrange("p (b j) -> p b j", j=K16 + 1)
            for j in range(K16):
                sr = grp_pool.tile([128, N], BF16, tag="srj")
                nc.vector.tensor_scalar(sr[:], iota_n[:], idx_f[:, j:j + 1], None,
                                        op0=ALU.is_equal)
                st_ps = ps_tr.tile([128, N], BF16, tag="tr")
                nc.tensor.transpose(st_ps[:, 0:128], sr[:, 0:128], ident_b[:])
                nc.tensor.transpose(st_ps[:, 128:256], sr[:, 128:256], ident_b[:])
                nc.scalar.copy(s_a_v[:, :, j], st_ps[:, 0:128])
                nc.scalar.copy(s_b_v[:, :, j], st_ps[:, 128:256])
            st_ps = ps_tr.tile([128, N], BF16, tag="tr")
            nc.tensor.transpose(st_ps[:, 0:128], score_bf[:, 0:128], ident_b[:])
            nc.tensor.transpose(st_ps[:, 128:256], score_bf[:, 128:256], ident_b[:])
            nc.scalar.copy(s_a_v[:, :, K16], st_ps[:, 0:128])
            nc.scalar.copy(s_b_v[:, :, K16], st_ps[:, 128:256])

            # stage 3: per-ba [gathered rows | u] = ve^T @ [one-hot | score]
            xq_sb = grp_pool.tile([128, GRP * (K16 + 1)], BF16, tag="xq")
            u_f = grp_pool.tile([128, GRP], F32, tag="uf")
            CH = 30
            xsel_ps = None
            for col in range(GRP):
                pos = col % CH
                if pos == 0:
                    xsel_ps = ps_xsel.tile([128, 510], F32, tag="xsel")
                vebf = venat_bf_g[col]
                lo, hi = pos * 17, pos * 17 + 17
                nc.tensor.matmul(xsel_ps[:, lo:hi], vebf[:, 0:128],
                                 s_a[:, col * 17:(col + 1) * 17],
                                 start=True, stop=False)
                nc.tensor.matmul(xsel_ps[:, lo:hi], vebf[:, 128:256],
                                 s_b[:, col * 17:(col + 1) * 17],
                                 start=False, stop=True)
                if pos == CH - 1 or col == GRP - 1:
                    c0 = (col // CH) * CH
                    cnt = col + 1 - c0
                    nc.scalar.copy(xq_sb[:, c0 * 17:(col + 1) * 17],
                                   xsel_ps[:, 0:cnt * 17])
                    xv = xsel_ps[:].rearrange("p (b j) -> p b j", j=K16 + 1)
                    nc.scalar.copy(u_f[:, c0:col + 1], xv[:, 0:cnt, K16])

            xq_v = xq_sb[:].rearrange("p (b j) -> p b j", j=K16 + 1)

            # vC = relu(Wfwd @ [vs | gathered] + bfwd)
            vc_ps = ps_tr.tile([128, GRP], F32, tag="tr")
            nc.tensor.matmul(vc_ps[:], wfT_b[:, 0:H], vst_b[g][:],
                             start=True, stop=False)
            for j in range(1, K16 + 1):
                nc.tensor.matmul(vc_ps[:], wfT_b[:, j * H:(j + 1) * H],
                                 xq_v[:, :, j - 1],
                                 start=False, stop=(j == K16))
            vc_sb = grp_pool.tile([128, GRP], F32, tag="vcsb")
            nc.scalar.activation(vc_sb[:], vc_ps[:], ACTF.Relu,
                                 bias=bfwd[:], scale=1.0)
            vc_tp = ps_tr.tile([128, GRP], F32, tag="tr")
            nc.tensor.transpose(vc_tp[:], vc_sb[:], ident_f[:])
            vc_rows = grp_pool.tile([GRP, H], F32, tag="vcrows")
            nc.scalar.copy(vc_rows[:], vc_tp[:])
            nc.sync.dma_start(vc_d.ap()[g * GRP:(g + 1) * GRP, :], vc_rows[:])

            # vM = relu(Wm0 @ vs + Wmv @ u + bmot)
            vm_ps = ps_tr.tile([128, GRP], F32, tag="tr")
            nc.tensor.matmul(vm_ps[:], wm0T[:], vst_f[g][:],
                             start=True, stop=False)
            nc.tensor.matmul(vm_ps[:], wmvT_f[:], u_f[:],
                             start=False, stop=True)
            vm_sb = grp_pool.tile([128, GRP], F32, tag="vmsb")
            nc.scalar.activation(vm_sb[:], vm_ps[:], ACTF.Relu,
                                 bias=bmot[:], scale=1.0)
            vm_tp = ps_tr.tile([128, GRP], F32, tag="tr")
            nc.tensor.transpose(vm_tp[:], vm_sb[:], ident_f[:])
            vm_rows = grp_pool.tile([GRP, H], F32, tag="vmrows")
            nc.scalar.copy(vm_rows[:], vm_tp[:])
            nc.sync.dma_start(vm_d.ap()[g * GRP:(g + 1) * GRP, :], vm_rows[:])


def _get_compiled(B_pc):
    key = B_pc
    if key not in _CACHE:
        nc = bacc.Bacc("TRN2", target_bir_lowering=False, debug=False,
                       num_devices=N_CORES)
        _build(nc, B_pc)
        nc.compile()
        _CACHE[key] = nc
    return _CACHE[key]


def kernel(vs, ve, ve_dead, Wq, Wk, Wv, W_mot, b_mot, W_fwd, b_fwd,
           trace=False, trace_kwargs=None):
    vs = np.asarray(vs, dtype=np.float32)
    ve = np.asarray(ve, dtype=np.float32)
    ve_dead = np.asarray(ve_dead, dtype=np.int32)
    Bq, Aq = vs.shape[0], vs.shape[1]
    assert (Bq, Aq) == (B, A), (Bq, Aq)
    B_pc = B // N_CORES
    NBA = B_pc * A

    nc = _get_compiled(B_pc)

    shared = {
        "wq": np.ascontiguousarray(Wq, dtype=np.float32),
        "wk": np.ascontiguousarray(Wk, dtype=np.float32),
        "wv": np.ascontiguousarray(Wv, dtype=np.float32),
        "wmot": np.ascontiguousarray(W_mot, dtype=np.float32),
        "bmot": np.ascontiguousarray(b_mot, dtype=np.float32).reshape(H, 1),
        "wfwd": np.ascontiguousarray(W_fwd, dtype=np.float32),
        "bfwd": np.ascontiguousarray(b_fwd, dtype=np.float32).reshape(H, 1),
    }
    in_maps = []
    for c in range(N_CORES):
        sl = slice(c * B_pc, (c + 1) * B_pc)
        in_maps.append({
            "ve": np.ascontiguousarray(ve[sl].reshape(NBA, N, H)),
            "vs": np.ascontiguousarray(vs[sl].reshape(NBA, H)),
            "dead": np.ascontiguousarray(ve_dead[sl].reshape(NBA, N)),
            **shared,
        })

    res = bass_utils.run_bass_kernel_spmd(
        nc, in_maps, core_ids=list(range(N_CORES)),
        trace=trace, **(trace_kwargs or {}))

    vc = np.empty((B, A, H), dtype=np.float32)
    vm = np.empty((B, A, H), dtype=np.float32)
    for c in range(N_CORES):
        sl = slice(c * B_pc, (c + 1) * B_pc)
        vc[sl] = res.results[c]["vc"].reshape(B_pc, A, H)
        vm[sl] = res.results[c]["vm"].reshape(B_pc, A, H)
    kernel.last_results = res
    return (vc, vm)
